# revision 16
# baseline (speedup 1.0000x reference)
"""Trainium2 Bass kernel for nn_Calculator_61993557950977.

Math: for each beta, k_beta = floor(1/(1-(1-1/beta)) - 1)  (== floor(beta-1)
up to f32 rounding).  The reference's [B, dim] masked reductions collapse to

    c_j = #{b : k_beta_b > j}             (reverse cumulative histogram)
    d_j = sum_b [k_beta_b > j] * log(k_beta_b)

    ixt   = sum_j gamma_j * (d_j - log(j+1) * c_j)
    n_I   = sum_j gamma_j * c_j
    G     = sum_j gamma_j * log(lambda_j) * c_j
    H     = sum_j gamma_j * log1p(-lambda_j) * c_j

(the reference's log-ratio telescopes to log(k_beta) - log(j+1)).

On device, with j = 128*q + s (q in [0,32), s in [0,128)) and per-beta
(qb, rb) = divmod(k_beta, 128), a single transposed-orientation PSUM
accumulation over 8 batch tiles produces everything:

    stationary[b, 1+s] = (s < rb_b), stationary[b, 0] = 1      [128, 128] bf16
    moving[b, :] = [onehot(qb) | onehot*lk_hi | onehot*lk_lo]  [128, 96] bf16

    psum[0,   32k+q] = hist[q] / histlog limbs                 (ones row)
    psum[1+s, q]     = Pc[q, s] = #{b: qb=q, rb>s}
    psum[1+s, 32k+q] = Pd limbs = sum lk*[qb=q][rb>s]

(lk = log(k_beta) split into bf16 hi+lo limbs so PE products stay exact in
f32 PSUM).  The j-space dot products then run with 128 partitions x 32 free:
GP = gammaT*Pc once, then one 3-block multiply against the transposed
[log(j+1) | log(lambda) | log1p(-lambda)] tables, plus gammaT*Pd limbs; a
[1,6] PE column-sum (ones stationary) collapses partitions so the outputs
are two single-packet DMAs ([1,96] hist row on the scalar queue, [1,6] dot
sums on the sync queue).  The host combines per-core partials (suffix sums
+ a handful of dots with table rowsums, f64).

Batch (8192) is sharded 1024 per core across 8 cores, 8 tiles of 128.
Index grids are int16; all four input DMAs ride the sync queue in
dependency order (betas first) so the scalar engine only runs the ACT
table load, Ln, and the hi-limb/row0 copies.
"""

import os
import sys

for _p in ("/opt/trn_rl_repo",):
    if os.path.isdir(_p) and _p not in sys.path:
        sys.path.insert(0, _p)

import numpy as np

# Module constants from the reference nn.Module
IXY = 1.0
HX = 10.0
ALPHA = 2.0
C = 1.0
DIM = 4096
B = 8192

N_CORES = 8
BS = B // N_CORES          # betas per core
NT = BS // 128             # 8 batch tiles of 128 per core
NQ = 32                    # coarse bins  (DIM = NQ * GR)
GR = 128                   # fine bins per coarse bin
NVT = 4                    # step-mask tiles in the first (PE-feeding) half

_CACHE = {}


def _build_nc(surgery=True):
    import concourse.bacc as bacc
    import concourse.bass as bass
    import concourse.tile as tile
    from concourse import mybir

    f32 = mybir.dt.float32
    i16 = mybir.dt.int16
    bf16 = mybir.dt.bfloat16
    Alu = mybir.AluOpType
    ACT = mybir.ActivationFunctionType
    AX = mybir.AxisListType

    nc = bacc.Bacc("TRN2", target_bir_lowering=False, debug=False)

    # bin: [8,136] = betas rows | 8x8 identity
    bin_t = nc.dram_tensor("bin", [8, GR + 8], f32, kind="ExternalInput")
    # ci: [128,160] int16 = iq grid (0..31) | ir grid (-1..126)
    ci_t = nc.dram_tensor("ci", [GR, NQ + GR], i16, kind="ExternalInput")
    # tb1: [128,34] = gammaT | 0.0 col | 1.0 col
    tb1_t = nc.dram_tensor("tb1", [GR, NQ + 2], f32, kind="ExternalInput")
    # tb2: [128,96] = lnjT | ln(lambda)T | log1p(-lambda)T  (rows shifted so
    # row 0 pairs with the all-ones stationary column and is zero)
    tb2_t = nc.dram_tensor("tb2", [GR, 3 * NQ], f32, kind="ExternalInput")
    o6_t = nc.dram_tensor("o6", [1, 6], f32, kind="ExternalOutput")
    orow_t = nc.dram_tensor("orow", [1, 3 * NQ], f32, kind="ExternalOutput")

    def with_mid(ap, pair):
        # [P, ...] -> [P, pair, ...] inserting a (stride, size) dim after P
        return bass.AP(tensor=ap.tensor, offset=ap.offset,
                       ap=[ap.ap[0], pair] + list(ap.ap[1:]))

    def bc_mid(ap, n):
        return with_mid(ap, [0, n])

    def bc_last(ap, n):
        # [P, F] -> [P, F, n] with stride-0 last dim
        return bass.AP(tensor=ap.tensor, offset=ap.offset,
                       ap=[ap.ap[0], ap.ap[1], [0, n]])

    with tile.TileContext(nc) as tc:
        with tc.tile_pool(name="sb", bufs=1) as sb, \
             tc.tile_pool(name="ps", bufs=1, space="PSUM") as ps:
            # ---- inputs (all on the sync queue, betas first) ----
            bin8 = sb.tile([8, GR + 8], f32)
            nc.sync.dma_start(out=bin8, in_=bin_t[:, :])
            ci = sb.tile([GR, NQ + GR], i16)
            nc.sync.dma_start(out=ci, in_=ci_t[:, :])
            tb1 = sb.tile([GR, NQ + 2], f32)
            nc.sync.dma_start(out=tb1, in_=tb1_t[:, :])
            tb2 = sb.tile([GR, 3 * NQ], f32)
            nc.sync.dma_start(out=tb2, in_=tb2_t[:, :])

            iq_i = ci[:, 0:NQ]
            ir_i = ci[:, NQ:]                   # values -1..126
            gT = tb1[:, 0:NQ]
            zc = tb1[:, NQ:NQ + 1]              # 0.0 col
            oc = tb1[:, NQ + 1:NQ + 2]          # 1.0 col (also PE-sum ones)
            t3sl = tb2[:, :]
            T3 = bass.AP(tensor=t3sl.tensor, offset=t3sl.offset,
                         ap=[t3sl.ap[0], [NQ, 3], [1, NQ]])

            # ---- transpose betas to [128, NT] via the tensor engine ----
            beta_ps = ps.tile([GR, 8], f32)
            nc.tensor.transpose(beta_ps, bin8[:, 0:GR], bin8[:, GR:GR + 8])

            # ---- per-beta prep ([128, NT], int16) ----
            kh = sb.tile([128, NT], f32)
            kbi = sb.tile([128, NT], i16)
            rbi = sb.tile([128, NT], i16)
            qbi = sb.tile([128, NT], i16)
            lk = sb.tile([128, NT], f32)
            limb = sb.tile([128, NT, 2], bf16)
            S = sb.tile([128, NT, GR], bf16)
            M = sb.tile([128, 3, NT, NQ], bf16)   # block-major: oh|oh*hi|oh*lo
            with tc.high_priority():
                # k_beta = floor(beta-1) via RNE int writeback of (beta-1.5)
                # (two steps: int16 writeback is rejected for PSUM sources)
                nc.vector.tensor_scalar(kh, beta_ps, 1.0, None,
                                        op0=Alu.subtract)
                nc.vector.tensor_scalar(kbi, kh, 0.5, None,
                                        op0=Alu.subtract)
                nc.vector.tensor_scalar(rbi, kbi, 127, None,
                                        op0=Alu.bitwise_and)
                # q = floor(k/128) via RNE((beta-1)/128 - 0.5): int16 shifts
                # fail the ISA check, but k/128 is exact in f32
                nc.vector.tensor_scalar(qbi, kh, 1.0 / 128.0, 0.5,
                                        op0=Alu.mult, op1=Alu.subtract)
                # stationary step masks: S[:, t, 1+s] = (s < rb), col 0 = 1
                nc.vector.tensor_tensor(
                    S[:, 0:NVT, :], bc_mid(ir_i, NVT),
                    bc_last(rbi[:, 0:NVT], GR), op=Alu.is_lt)
                nc.vector.tensor_tensor(M[:, 0, :, :], bc_mid(iq_i, NT),
                                        bc_last(qbi, NQ), op=Alu.is_equal)
                nc.scalar.activation(out=lk, in_=kbi, func=ACT.Ln, bias=zc)
                nc.scalar.activation(out=limb[:, :, 0], in_=lk, func=ACT.Copy,
                                     bias=0.0)                # hi limb
                nc.vector.tensor_tensor(limb[:, :, 1], lk, limb[:, :, 0],
                                        op=Alu.subtract)      # lo limb
                # M[:, 1+l, t, q] = onehot * limb_l  (both limbs at once)
                o_dst = M[:, 1:3, :, :]
                o_src = bc_mid(M[:, 0, :, :], 2)
                lf = limb[:, :, :]
                l_src = bass.AP(tensor=lf.tensor, offset=lf.offset,
                                ap=[lf.ap[0], [1, 2], [2, NT], [0, NQ]])
                nc.vector.tensor_tensor(o_dst, o_src, l_src, op=Alu.mult)
                nc.vector.tensor_tensor(
                    S[:, NVT:NT, :], bc_mid(ir_i, NT - NVT),
                    bc_last(rbi[:, NVT:NT], GR), op=Alu.is_lt)

            # ---- single PSUM accumulation over the 8 batch tiles ----
            psum = ps.tile([GR, 3 * NQ], f32)
            for t in range(NT):
                nc.tensor.matmul(psum, S[:, t, :], M[:, :, t, :],
                                 start=(t == 0), stop=(t == NT - 1))

            # row 0 of psum = [hist | histlog_hi | histlog_lo]: scalar copies
            # it and ships it on the otherwise-idle scalar DMA queue while
            # the vector engine runs the dot products
            orow = sb.tile([1, 3 * NQ], f32)
            nc.scalar.activation(out=orow, in_=psum[0:1, :], func=ACT.Copy,
                                 bias=0.0)
            nc.scalar.dma_start(out=orow_t[:, :], in_=orow)

            # ---- dot products against Pc / Pd (vector reads PSUM) ----
            # P6 blocks: 0=E2', 1=G', 2=H', 3=Nn' (=GP), 4:6=E1' limbs
            P6 = sb.tile([GR, 6, NQ], f32)
            GP = P6[:, 3, :]
            nc.vector.tensor_tensor(GP, gT, psum[:, 0:NQ], op=Alu.mult)
            nc.vector.tensor_tensor(P6[:, 0:3, :], T3, bc_mid(GP, 3),
                                    op=Alu.mult)
            pd = with_mid(psum[:, NQ:2 * NQ], [NQ, 2])
            nc.vector.tensor_tensor(P6[:, 4:6, :], bc_mid(gT, 2), pd,
                                    op=Alu.mult)
            o6sb = sb.tile([GR, 6], f32)
            nc.vector.tensor_reduce(o6sb, P6, axis=AX.X, op=Alu.add)
            # collapse partitions on the PE: psum6[0, c] = sum_p o6sb[p, c]
            psum6 = ps.tile([1, 6], f32)
            nc.tensor.matmul(psum6, oc, o6sb, start=True, stop=True)
            o6out = sb.tile([1, 6], f32)
            nc.vector.tensor_copy(o6out, psum6)
            nc.sync.dma_start(out=o6_t[:, :], in_=o6out)

    nc.compile()
    if surgery:
        _surgery(nc)
    return nc


def _surgery(nc):
    """Post-compile stream surgery:
    - drop const-AP memsets and the all-engine entry barrier from the main
      block (body ordering is fully semaphore-protected; the entry/exit
      barriers each consume exactly what they produce on their semaphores,
      so the exit barrier still works);
    - hoist the input DMA dispatches to the head of the body block so their
      doorbells ring before the scalar engine's ACT table loads;
    - drop the exit-block's leading DMA-completion waits (nothing on device
      consumes the output DMAs; their semaphores are write-only) and the
      second exit barrier after the semaphore range-clear (the NEFF's own
      final all-engine rendezvous follows immediately).
    """
    f = nc.m.functions[0]
    main = f.blocks[0]
    main.instructions = [
        i for i in main.instructions
        if type(i).__name__ not in ("InstMemset", "InstDrain",
                                    "InstEventSemaphore")]
    body = f.blocks[1]

    def is_input_dma(i):
        if type(i).__name__ != "InstDMACopy" or not i.ins:
            return False
        return getattr(i.ins[0], "memref", None) in ("bin", "ci", "tb1", "tb2")

    front = [i for i in body.instructions if is_input_dma(i)]
    rest = [i for i in body.instructions if not is_input_dma(i)]
    assert len(front) == 4, f"expected 4 input DMAs, found {len(front)}"
    body.instructions = front + rest

    end = f.blocks[2]
    insts = list(end.instructions)
    i = 0
    while i < len(insts) and type(insts[i]).__name__ == "InstEventSemaphore":
        i += 1
    insts = insts[i:]
    isa = [j for j, x in enumerate(insts) if type(x).__name__ == "InstISA"]
    if isa:
        insts = insts[:isa[-1] + 1]
    end.instructions = insts


def run_device(betas, lambdas, gammas, trace=False):
    from concourse.bass_utils import run_bass_kernel_spmd

    if "nc" not in _CACHE:
        _CACHE["nc"] = _build_nc()
    nc = _CACHE["nc"]

    betas = np.ascontiguousarray(np.asarray(betas, dtype=np.float32).reshape(B))
    lambdas = np.asarray(lambdas, dtype=np.float32).reshape(DIM)
    gammas = np.asarray(gammas, dtype=np.float32).reshape(DIM)
    l64 = lambdas.astype(np.float64)
    lnj = np.log(np.arange(1, DIM + 1, dtype=np.float64))

    # transposed tables with the ones-row (s'=-1) slot zeroed
    def tshift(v):
        out = np.zeros((GR, NQ), np.float32)
        out[1:, :] = np.asarray(v, np.float64).reshape(NQ, GR)[:, 0:GR - 1].T
        return out

    tb1 = np.concatenate([
        tshift(gammas),
        np.zeros((GR, 1), np.float32), np.ones((GR, 1), np.float32)], axis=1)
    tb1 = np.ascontiguousarray(tb1)
    tb2 = np.ascontiguousarray(np.concatenate(
        [tshift(lnj), tshift(np.log(l64)), tshift(np.log1p(-l64))], axis=1))
    iq = np.broadcast_to(np.arange(NQ, dtype=np.int16), (GR, NQ))
    ir = np.broadcast_to(np.arange(-1, GR - 1, dtype=np.int16), (GR, GR))
    ci = np.ascontiguousarray(np.concatenate([iq, ir], axis=1))

    in_maps = []
    for i in range(N_CORES):
        bn = np.zeros((8, GR + 8), np.float32)
        bn[:, 0:GR] = betas[i * BS:(i + 1) * BS].reshape(8, GR)
        bn[:, GR:GR + 8] = np.eye(8, dtype=np.float32)
        in_maps.append({"bin": bn, "ci": ci, "tb1": tb1, "tb2": tb2})

    last_err = None
    res = None
    for _attempt in range(3):
        try:
            res = run_bass_kernel_spmd(nc, in_maps, core_ids=list(range(N_CORES)),
                                       trace=trace)
            break
        except Exception as e:  # transient device-recovery errors
            last_err = e
            res = None
    if res is None:
        raise last_err

    orow = np.stack([np.asarray(r["orow"], dtype=np.float64).reshape(3 * NQ)
                     for r in res.results])
    d6 = np.stack([np.asarray(r["o6"], dtype=np.float64).reshape(6)
                   for r in res.results])   # [cores,6]: E2 G H Nn E1hi E1lo
    hist = orow[:, 0:NQ]
    hlog = orow[:, NQ:2 * NQ] + orow[:, 2 * NQ:3 * NQ]
    Cq = np.cumsum(hist[:, ::-1], axis=1)[:, ::-1] - hist   # exclusive suffix
    Dq = np.cumsum(hlog[:, ::-1], axis=1)[:, ::-1] - hlog
    # beta-independent table rowsums (host, f64)
    g64 = gammas.astype(np.float64)
    rs_lnj = (g64 * lnj).reshape(NQ, GR).sum(1)
    rs_g = g64.reshape(NQ, GR).sum(1)
    rs_lnl = (g64 * np.log(l64)).reshape(NQ, GR).sum(1)
    rs_ln1m = (g64 * np.log1p(-l64)).reshape(NQ, GR).sum(1)
    E2 = d6[:, 0].sum() + (Cq * rs_lnj).sum()
    G = d6[:, 1].sum() + (Cq * rs_lnl).sum()
    H = d6[:, 2].sum() + (Cq * rs_ln1m).sum()
    Nn = d6[:, 3].sum() + (Cq * rs_g).sum()
    E1 = (d6[:, 4] + d6[:, 5]).sum() + (Dq * rs_g).sum()
    sums = (E1, E2, Nn, G, H)
    return sums, res


def _finalize(E1, E2, Nn, G, H):
    ixt = E1 - E2
    n_I = Nn
    gm_term = np.exp(G / n_I)
    gm_comp = np.exp(H / n_I)
    exp_term = np.exp(2.0 * ixt / n_I)
    log_term = -n_I / 2.0 * np.log(gm_comp + exp_term * gm_term)
    ity = ixt + log_term
    rhs = 1.0 - ity / IXY
    lhs_1 = 1.0 - ixt / HX
    if lhs_1 < 0:
        lhs_1 = abs(lhs_1) * 20.0
    lhs = C * lhs_1 ** ALPHA
    return (np.asarray(np.float32(rhs)), np.asarray(np.float32(lhs)))


def kernel(betas, lambdas, gammas):
    sums, _ = run_device(betas, lambdas, gammas, trace=False)
    return _finalize(*sums)
